# revision 12
# baseline (speedup 1.0000x reference)
"""Trainium2 Bass kernel for nn_DiscriminatorModel (8-layer MLP with
LayerNorm+LeakyReLU, 524288x128 input, data-parallel over 8 NeuronCores).

Algorithm (validated vs the jax reference to ~7e-4 relative absmax):
  - Mean-centering of each LayerNorm is folded into the weights host-side:
    Wc_l = W_l @ (I - 11^T/d)  => matmul output is already centered.
  - LayerNorm gammas are folded into the NEXT layer's weights host-side
    (LReLU(g*z) = g*LReLU(z) for g > 0), so the on-device activation is a
    plain LeakyReLU with no per-feature scale.
  - The per-row rsqrt(var+eps) scales commute through LeakyReLU and the
    following matmul; only the layer-6/7 variances matter to fp32 precision:
        E8 = v7 + eps*v6,   out = (a7 @ W8) / sqrt(E8) + b8
  - fp32-grade precision via fp16 multi-word matmuls (3 terms):
        z = Wh@ah + Wh@al + Wl@ah,  fp32 PSUM accumulate.
  - x is shipped as fp16 hi (2B) + e4m3 fp8 lo scaled by 2^12 (1B) =
    3 B/elem of HBM traffic; the lo word is upcast to fp16 on-device.
  - Activations are packed feature-major: 128 partitions = c blocks x dout
    features, rows along the free dim. Col/row tile_position packing keeps
    concurrent matmuls on the PE array. Per layer the activation split is
    three elementwise passes spread across ScalarE/VectorE/GpSimd:
        A: af32 = LReLU(psum)   B: ah = fp16(af32)   C: al = af32 - ah

Requires all LayerNorm beta == 0 and gamma > 0 (true for the reference
inputs); otherwise falls back to a numpy forward pass.
"""

import numpy as np
import ml_dtypes

EPS = 1e-5
SLOPE = 0.2
DIMS = [128, 32, 64, 32, 16, 8, 4, 2]
N_CORES = 8
ROWS = 524288
RPC = ROWS // N_CORES        # 65536 rows per core
R_ST = 8192                  # rows per supertile
N_ST = RPC // R_ST           # 8 supertiles per core
F16 = np.float16
F8LO = ml_dtypes.float8_e3m4
XLO_SCALE = 2.0 ** 12

_CACHE = {}


def _lrelu(x):
    return np.where(x > 0, x, SLOPE * x).astype(np.float32)


def _center(W):
    d = W.shape[1]
    return (np.asarray(W, np.float64) @ (np.eye(d) - 1.0 / d))


def _split(a):
    hi = a.astype(F16)
    lo = (a.astype(np.float32) - hi.astype(np.float32)).astype(F16)
    return hi, lo


def _blockdiag(W, c):
    din, dout = W.shape
    out = np.zeros((c * din, c * dout), W.dtype)
    for b in range(c):
        out[b * din:(b + 1) * din, b * dout:(b + 1) * dout] = W
    return out


def _transition_stat(W, c_in):
    """Parity-interleaved stationary for a c_in -> 2*c_in packing transition.

    Two stats (par=0,1), each [128, 128]: out col m = blk_out*w + f where
    w = 128/(2*c_in) per-block output width; nonzero iff blk_out % 2 == par,
    source block g = blk_out // 2 maps rows g*din..(g+1)*din <- W[:, f].
    """
    din, dout = W.shape
    w = 128 // (2 * c_in)
    assert w == dout
    stats = []
    for par in range(2):
        S = np.zeros((128, 128), W.dtype)
        for m in range(128):
            blk_out, f = divmod(m, w)
            if blk_out % 2 != par:
                continue
            g = blk_out // 2
            S[g * din:(g + 1) * din, m] = W[:, f]
        stats.append(S)
    return stats


def _var_stats(dout6, dout7):
    # V6 par-stats: s6 is 32-packed (32 blocks x 4 feats); v6' is 64 blocks.
    V6 = []
    for par in range(2):
        S = np.zeros((128, 64), np.float32)
        for m in range(64):
            if m % 2 != par:
                continue
            g = m // 2
            S[g * dout6:(g + 1) * dout6, m] = 1.0 / dout6
        V6.append(S)
    V7 = np.zeros((128, 64), np.float32)
    for m in range(64):
        V7[m * dout7:(m + 1) * dout7, m] = 1.0 / dout7
    return V6[0], V6[1], V7


def _numpy_forward(inp):
    h = np.asarray(inp["x"], np.float32)
    for i in range(7):
        W = np.asarray(inp[f"W{i+1}"], np.float32)
        g = np.asarray(inp[f"g{i+1}"], np.float32)
        b = np.asarray(inp[f"bt{i+1}"], np.float32)
        h = h @ W
        m = h.mean(-1, keepdims=True)
        v = np.square(h - m).mean(-1, keepdims=True)
        h = (h - m) / np.sqrt(v + EPS) * g + b
        h = _lrelu(h)
    return (h @ np.asarray(inp["W8"], np.float32)
            + np.asarray(inp["b8"], np.float32)).astype(np.float32)


def _build_consts(inp):
    """Host-side weight prep (gamma folded into next W). Returns fp16 pack."""
    gs = [np.asarray(inp[f"g{l}"], np.float64) for l in range(1, 8)]
    Ws = [np.asarray(inp[f"W{l}"], np.float64) for l in range(1, 8)]
    # fold gamma_{l-1} into W_l rows; gamma_7 into W8
    Wf = [Ws[0]]
    for i in range(1, 7):
        Wf.append(np.diag(gs[i - 1]) @ Ws[i])
    W8f = (np.diag(gs[6]) @ np.asarray(inp["W8"], np.float64)).astype(np.float32)
    Wc = [_center(Wf[i]).astype(np.float32) for i in range(7)]

    cols = {}
    def add(name, arr32):
        hi, lo = _split(arr32)
        cols[name + "h"], cols[name + "l"] = hi, lo

    # L1 stationaries pre-scaled by XLO_SCALE (exact power of 2): the xlo
    # fp8 word is upcast WITHOUT descaling (pure cast), so all three L1
    # terms come out scaled by XLO_SCALE; the L1 activation descales via
    # its scale param. The Wh@xlo term needs the UNSCALED hi stationary.
    add("s1", Wc[0] * XLO_SCALE)                       # [128, 32]
    cols["s1u"] = _split(Wc[0])[0]                     # unscaled hi
    # L2 row-tiled: blockdiag2(Wc2) [64,128] stacked twice -> [128,128]
    bd2 = _blockdiag(Wc[1], 2)
    add("s2", np.vstack([bd2, bd2]))
    add("s3", _blockdiag(Wc[2], 2))                    # [128, 64]
    for l, c_in in ((4, 4), (5, 8), (6, 16), (7, 32)):
        t0, t1 = _transition_stat(Wc[l - 1], c_in)
        add(f"t{l}a", t0)
        add(f"t{l}b", t1)
    add("s8", _blockdiag(W8f, 64))                     # [128, 64]

    # pack all fp16 stationaries into one [128, T] array; remember offsets
    order = sorted(cols.keys())
    offs, total = {}, 0
    for k in order:
        offs[k] = total
        total += cols[k].shape[1]
    wpack = np.zeros((128, total), F16)
    for k in order:
        wpack[:, offs[k]:offs[k] + cols[k].shape[1]] = cols[k]

    V6a, V6b, V7 = _var_stats(DIMS[6], DIMS[7])
    vpack = np.concatenate([V6a, V6b, V7], axis=1).astype(np.float32)
    return wpack, offs, vpack


def _split_multi_waits(nc):
    """Walrus build limit: <=1 sync wait per instruction. Hoist extras onto
    same-engine NOPs inserted just before the instruction."""
    import concourse.mybir as mybir
    import bass_rust
    cnt = 0
    for f in nc.m.functions:
        for blk in f.blocks:
            newlist = []
            for inst in blk.instructions:
                si = inst.sync_info
                waits = list(si.on_wait) if si is not None and si.on_wait else []
                if len(waits) > 1:
                    for w in waits[:-1]:
                        nop = mybir.InstNoOp(name=f"waitnop_{cnt}", ins=[], outs=[])
                        cnt += 1
                        nop.engine = inst.engine
                        nop.sync_info = bass_rust.SyncInfo(on_wait=[w], on_update=[])
                        newlist.append(nop)
                    inst.sync_info = bass_rust.SyncInfo(
                        on_wait=[waits[-1]], on_update=list(si.on_update))
                newlist.append(inst)
            blk.instructions = newlist
    return cnt


def _build_program(offs, wpack_cols, b8_val):
    import concourse.bass as bass
    import concourse.mybir as mybir
    from concourse.tile import TileContext
    from contextlib import ExitStack

    # this walrus build rejects >1 sync wait on the tail Drain; split them
    import bass_rust
    from concourse.tile import TileContext as _TC
    from concourse.vector_clock import ScopedClock

    def _patched_drain(self, tick_clock, wait_clock):
        probe = self.nc.sync.nop()
        wait_clock.add_sem_waits(probe.ins,
                                 ScopedClock({None: tick_clock.global_clock}))
        si = probe.ins.sync_info
        waits = list(si.on_wait) if si is not None else []
        upd = list(si.on_update) if si is not None else []
        probe.ins.sync_info = bass_rust.SyncInfo(on_wait=waits[:1], on_update=upd)
        for w in waits[1:]:
            nop = self.nc.sync.nop()
            nop.ins.sync_info = bass_rust.SyncInfo(on_wait=[w], on_update=[])
        self.nc.sync.drain()
        self.nc.all_engine_barrier()
        assert self.sems is not None
        popped = self.nc._tile_sem_poison_stack.pop()
        assert popped is self._sem_poison
        self.nc.clear_and_free_semaphores(list(self.sems.allocated().values()))
        self.nc.all_engine_barrier()

    _TC._drain_and_barrier = _patched_drain

    f16, f32 = mybir.dt.float16, mybir.dt.float32
    f8 = mybir.dt.float8e3
    AF = mybir.ActivationFunctionType
    OP = mybir.AluOpType

    nc = bass.Bass(trn_type="TRN2", num_swdge_queues=4)
    xhi_d = nc.dram_tensor("xhi", [128, RPC], f16, kind="ExternalInput")
    xlo_d = nc.dram_tensor("xlo", [128, RPC], f8, kind="ExternalInput")
    wp_d = nc.dram_tensor("wpack", [128, wpack_cols], f16, kind="ExternalInput")
    vp_d = nc.dram_tensor("vpack", [128, 192], f32, kind="ExternalInput")
    out_d = nc.dram_tensor("out", [N_ST * 64, R_ST // 64], f32,
                           kind="ExternalOutput")

    with TileContext(nc) as tc:
        with ExitStack() as ctx:
            const = ctx.enter_context(tc.tile_pool(name="const", bufs=1))
            wp = const.tile([128, wpack_cols], f16)
            nc.sync.dma_start(wp[:, :], wp_d[:, :])
            vpk = const.tile([128, 192], f32)
            nc.sync.dma_start(vpk[:, :], vp_d[:, :])

            def W(name):
                return wp[:, offs[name]:offs[name] + _WCOLS[name]]

            xp = ctx.enter_context(tc.tile_pool(name="xp", bufs=2))
            ap = ctx.enter_context(tc.tile_pool(name="ap", bufs=2))
            fin = ctx.enter_context(tc.tile_pool(name="fin", bufs=N_ST))
            up = ctx.enter_context(tc.tile_pool(name="up", bufs=2, space="PSUM"))
            vp = ctx.enter_context(tc.tile_pool(name="vp", bufs=3, space="PSUM"))

            def mm(out, lhsT, rhs, start, stop, tp=None):
                # matmul output must fit one PSUM bank: 512 fp32 columns
                n = out.shape[1]
                for o in range(0, n, 512):
                    e = min(o + 512, n)
                    nc.tensor.matmul(out[:, o:e], lhsT, rhs[:, o:e],
                                     start=start, stop=stop, tile_position=tp)

            ysbs, e8sbs = [], []

            for st in range(N_ST):
                x0 = st * R_ST
                xh, xl = [], []
                dma_engs = [nc.sync, nc.gpsimd, nc.scalar, nc.gpsimd]
                for k in range(2):
                    xht = xp.tile([128, 4096], f16, name=f"xh{k}")
                    dma_engs[k].dma_start(
                        xht[:, :],
                        xhi_d[:, x0 + 4096 * k:x0 + 4096 * (k + 1)])
                    xh.append(xht)
                    xlt = xp.tile([128, 4096], f8, name=f"xl8{k}")
                    dma_engs[2 + k].dma_start(
                        xlt[:, :],
                        xlo_d[:, x0 + 4096 * k:x0 + 4096 * (k + 1)])
                    # pure-cast upcast of the fp8 lo word (still x2^12 scaled)
                    xlt16 = xp.tile([128, 4096], f16, name=f"xl{k}")
                    nc.vector.tensor_copy(xlt16[:, :], xlt[:, :])
                    xl.append(xlt16)

                def act_split(u, n, hi, lo, col0, scale=1.0):
                    """A: af32=LReLU(psum) on ScalarE; B: fp16 cast on VectorE;
                    C: lo residual on GpSimd."""
                    af = ap.tile([128, 1024], f32, name="af", tag="af", bufs=4)
                    afv = af[:, :n]
                    nc.scalar.activation(afv, u[:, :n], AF.Prelu,
                                         bias=0.0, scale=scale, alpha=SLOPE)
                    nc.vector.tensor_copy(hi[:, col0:col0 + n], afv)
                    nc.vector.tensor_tensor(lo[:, col0:col0 + n], afv,
                                            hi[:, col0:col0 + n], OP.subtract)

                # ---- L1: u1 [128, 2048] (c=4), 2 psum chunks
                a1h = ap.tile([128, 2048], f16)
                a1l = ap.tile([128, 2048], f16)
                for c in range(2):
                    u = up.tile([128, 1024], f32, name="u", tag="u")
                    # term-outer order: adjacent mms hit different col-groups
                    # so they run concurrently on disjoint PE subarrays
                    for t in range(3):
                        for b in range(4):
                            rh = xh[b // 2][:, (b % 2) * 2048 + 1024 * c:][:, :1024]
                            rl = xl[b // 2][:, (b % 2) * 2048 + 1024 * c:][:, :1024]
                            S = W(("s1h", "s1u", "s1l")[t])
                            r = rh if t != 1 else rl
                            mm(u[32 * b:32 * (b + 1), :], S, r,
                               start=(t == 0), stop=(t == 2), tp=(0, 32 * b))
                    act_split(u, 1024, a1h, a1l, 1024 * c, scale=1.0 / XLO_SCALE)

                # ---- L2 row-tiled: two tensors u2_q [128, 2048]
                a2h = [ap.tile([128, 2048], f16, name=f"a2h{q}") for q in range(2)]
                a2l = [ap.tile([128, 2048], f16, name=f"a2l{q}") for q in range(2)]
                s2hs = [wp[:, offs["s2h"]:offs["s2h"] + 128][64 * q:64 * (q + 1), :]
                        for q in range(2)]
                s2ls = [wp[:, offs["s2l"]:offs["s2l"] + 128][64 * q:64 * (q + 1), :]
                        for q in range(2)]
                for c in range(2):
                    us = [up.tile([128, 1024], f32, name="u", tag="u")
                          for _ in range(2)]
                    for t in range(3):
                        for q in range(2):
                            rh = a1h[64 * q:64 * (q + 1), 1024 * c:1024 * (c + 1)]
                            rl = a1l[64 * q:64 * (q + 1), 1024 * c:1024 * (c + 1)]
                            S = s2hs[q] if t < 2 else s2ls[q]
                            r = rh if t != 1 else rl
                            mm(us[q][:, :], S, r, start=(t == 0),
                               stop=(t == 2), tp=(64 * q, 0))
                    for q in range(2):
                        act_split(us[q], 1024, a2h[q], a2l[q], 1024 * c)

                # ---- L3 col-tiled: u3 [128, 2048] (c=4)
                a3h = ap.tile([128, 2048], f16)
                a3l = ap.tile([128, 2048], f16)
                for c in range(2):
                    u = up.tile([128, 1024], f32, name="u", tag="u")
                    for t in range(3):
                        for q in range(2):
                            rh = a2h[q][:, 1024 * c:1024 * (c + 1)]
                            rl = a2l[q][:, 1024 * c:1024 * (c + 1)]
                            S = W("s3h") if t < 2 else W("s3l")
                            r = rh if t != 1 else rl
                            mm(u[64 * q:64 * (q + 1), :], S, r,
                               start=(t == 0), stop=(t == 2), tp=(0, 64 * q))
                    act_split(u, 1024, a3h, a3l, 1024 * c)

                # ---- L4..L7: parity transitions, halving free size
                prev_h, prev_l = a3h, a3l
                n_prev = 2048
                s6 = s7 = None
                for li, l in enumerate((4, 5, 6, 7)):
                    n = n_prev // 2
                    u = up.tile([128, 1024], f32, name="u", tag="u")
                    uv = u[:, :n]
                    first = True
                    for par, suf in ((0, "a"), (1, "b")):
                        rh = prev_h[:, par * n:(par + 1) * n]
                        rl = prev_l[:, par * n:(par + 1) * n]
                        th, tl = W(f"t{l}{suf}h"), W(f"t{l}{suf}l")
                        mm(uv, th, rh, start=first, stop=False)
                        mm(uv, th, rl, start=False, stop=False)
                        mm(uv, tl, rh, start=False, stop=(par == 1))
                        first = False
                    nh = ap.tile([128, n], f16, name=f"a{l}h")
                    nl = ap.tile([128, n], f16, name=f"a{l}l")
                    if l == 6:
                        # fp32 squares (fp16 would underflow degenerate rows)
                        s6 = ap.tile([128, 256], f32)
                        nc.scalar.activation(s6[:, :], uv, AF.Square)
                    if l == 7:
                        s7 = ap.tile([128, 128], f32)
                        nc.scalar.activation(s7[:, :], uv, AF.Square)
                    act_split(u, n, nh, nl, 0)
                    prev_h, prev_l, n_prev = nh, nl, n

                # ---- variances (fp32: degenerate rows need the range)
                v6t = vp.tile([64, 128], f32, name="v6t", tag="v")
                mm(v6t[:, :], vpk[:, 0:64], s6[:, 0:128],
                   start=True, stop=False)
                mm(v6t[:, :], vpk[:, 64:128], s6[:, 128:256],
                   start=False, stop=True)
                v7t = vp.tile([64, 128], f32, name="v7t", tag="v")
                mm(v7t[:, :], vpk[:, 128:192], s7[:, :],
                   start=True, stop=True)

                # ---- L8: y = a7 @ blockdiag64(W8)
                yt = vp.tile([64, 128], f32, name="yt", tag="v")
                mm(yt[:, :], W("s8h"), prev_h[:, :], start=True, stop=False)
                mm(yt[:, :], W("s8h"), prev_l[:, :], start=False, stop=False)
                mm(yt[:, :], W("s8l"), prev_h[:, :], start=False, stop=True)

                # ---- stash y and E8 = v7 + eps*v6 (sqrt deferred)
                v7sb = fin.tile([64, 128], f32, name="v7sb", tag="v7sb", bufs=2)
                nc.scalar.copy(v7sb[:, :], v7t[:, :])
                ysb = fin.tile([64, 128], f32, name="ysb", tag="ysb")
                nc.scalar.copy(ysb[:, :], yt[:, :])
                e8 = fin.tile([64, 128], f32, name="e8", tag="e8")
                nc.vector.scalar_tensor_tensor(e8[:, :], v6t[:, :], EPS,
                                               v7sb[:, :], OP.mult, OP.add)
                ysbs.append(ysb)
                e8sbs.append(e8)

            # ---- final: out = y / sqrt(E8) + b8 (one Sqrt table-load)
            for st in range(N_ST):
                sq = fin.tile([64, 128], f32, name="sq", tag="sq", bufs=2)
                nc.scalar.activation(sq[:, :], e8sbs[st][:, :], AF.Sqrt)
                rinv = fin.tile([64, 128], f32, name="rinv", tag="rinv", bufs=2)
                nc.vector.reciprocal(rinv[:, :], sq[:, :])
                osb = fin.tile([64, 128], f32, name="osb", tag="osb", bufs=2)
                nc.vector.tensor_tensor(osb[:, :], ysbs[st][:, :], rinv[:, :],
                                        OP.mult)
                nc.vector.tensor_scalar(osb[:, :], osb[:, :], b8_val,
                                        None, OP.add)
                nc.sync.dma_start(out_d[st * 64:(st + 1) * 64, :], osb[:, :])
    _split_multi_waits(nc)
    return nc


_WCOLS = {}


def kernel(**inputs):
    for l in range(1, 8):
        if np.abs(np.asarray(inputs[f"bt{l}"], np.float32)).max() > 0:
            return _numpy_forward(inputs)
        if np.asarray(inputs[f"g{l}"], np.float32).min() <= 0:
            return _numpy_forward(inputs)

    wpack, offs, vpack = _build_consts(inputs)
    global _WCOLS
    _WCOLS = {"s1h": 32, "s1l": 32, "s1u": 32, "s2h": 128, "s2l": 128,
              "s3h": 64, "s3l": 64, "s8h": 64, "s8l": 64}
    for l in range(4, 8):
        for suf in ("a", "b"):
            _WCOLS[f"t{l}{suf}h"] = 128
            _WCOLS[f"t{l}{suf}l"] = 128

    x = np.asarray(inputs["x"], np.float32)
    xT = np.ascontiguousarray(x.T)               # [128, 524288]
    xhi = xT.astype(F16)
    xlo8 = ((xT - xhi.astype(np.float32)) * XLO_SCALE).astype(F8LO)
    b8 = np.asarray(inputs["b8"], np.float32).reshape(1, 1)

    nc = _build_program(offs, wpack.shape[1], float(b8[0, 0]))

    in_maps = []
    for c in range(N_CORES):
        s = slice(c * RPC, (c + 1) * RPC)
        in_maps.append({
            "xhi": np.ascontiguousarray(xhi[:, s]),
            "xlo": np.ascontiguousarray(xlo8[:, s]),
            "wpack": wpack, "vpack": vpack,
        })

    from concourse.bass_utils import run_bass_kernel_spmd
    res = run_bass_kernel_spmd(nc, in_maps, core_ids=list(range(N_CORES)))

    out = np.empty((ROWS, 1), np.float32)
    for c in range(N_CORES):
        out[c * RPC:(c + 1) * RPC, 0] = res.results[c]["out"].reshape(-1)
    return out


# revision 16
# speedup vs baseline: 1.0114x; 1.0114x over previous
"""Trainium2 Bass kernel for nn_DiscriminatorModel (8-layer MLP with
LayerNorm+LeakyReLU, 524288x128 input, data-parallel over 8 NeuronCores).

Algorithm (validated vs the jax reference to ~7e-4 relative absmax):
  - Mean-centering of each LayerNorm is folded into the weights host-side:
    Wc_l = W_l @ (I - 11^T/d)  => matmul output is already centered.
  - LayerNorm gammas are folded into the NEXT layer's weights host-side
    (LReLU(g*z) = g*LReLU(z) for g > 0), so the on-device activation is a
    plain LeakyReLU with no per-feature scale.
  - The per-row rsqrt(var+eps) scales commute through LeakyReLU and the
    following matmul; only the layer-6/7 variances matter to fp32 precision:
        E8 = v7 + eps*v6,   out = (a7 @ W8) / sqrt(E8) + b8
  - fp32-grade precision via fp16 multi-word matmuls (3 terms):
        z = Wh@ah + Wh@al + Wl@ah,  fp32 PSUM accumulate.
  - Activations are packed feature-major: 128 partitions = c blocks x dout
    features, rows along the free dim. Col/row tile_position packing keeps
    concurrent matmuls on the PE array.
  - Software pipelining: the serial L4->L7 ladder of supertile k is emitted
    interleaved with the wide L1-L3 matmuls of supertile k+1, so the PE
    never waits on the act chain; input DMA is prefetched 2 supertiles
    ahead. Per layer the activation split is three elementwise passes:
        A: af32 = LReLU(psum) [ScalarE]
        B: ah = fp16(af32)    [VectorE or GpSimd per layer]
        C: al = af32 - ah     [VectorE or GpSimd per layer]

Requires all LayerNorm beta == 0 and gamma > 0 (true for the reference
inputs); otherwise falls back to a numpy forward pass.
"""

import numpy as np

EPS = 1e-5
SLOPE = 0.2
DIMS = [128, 32, 64, 32, 16, 8, 4, 2]
N_CORES = 8
ROWS = 524288
RPC = ROWS // N_CORES        # 65536 rows per core
R_ST = 8192                  # rows per supertile
N_ST = RPC // R_ST           # 8 supertiles per core
F16 = np.float16

_CACHE = {}


def _lrelu(x):
    return np.where(x > 0, x, SLOPE * x).astype(np.float32)


def _center(W):
    d = W.shape[1]
    return (np.asarray(W, np.float64) @ (np.eye(d) - 1.0 / d))


def _split(a):
    hi = a.astype(F16)
    lo = (a.astype(np.float32) - hi.astype(np.float32)).astype(F16)
    return hi, lo


def _blockdiag(W, c):
    din, dout = W.shape
    out = np.zeros((c * din, c * dout), W.dtype)
    for b in range(c):
        out[b * din:(b + 1) * din, b * dout:(b + 1) * dout] = W
    return out


def _transition_stat(W, c_in):
    """Parity-interleaved stationary for a c_in -> 2*c_in packing transition.

    Two stats (par=0,1), each [128, 128]: out col m = blk_out*w + f where
    w = 128/(2*c_in) per-block output width; nonzero iff blk_out % 2 == par,
    source block g = blk_out // 2 maps rows g*din..(g+1)*din <- W[:, f].
    """
    din, dout = W.shape
    w = 128 // (2 * c_in)
    assert w == dout
    stats = []
    for par in range(2):
        S = np.zeros((128, 128), W.dtype)
        for m in range(128):
            blk_out, f = divmod(m, w)
            if blk_out % 2 != par:
                continue
            g = blk_out // 2
            S[g * din:(g + 1) * din, m] = W[:, f]
        stats.append(S)
    return stats


def _var_stats(dout6, dout7):
    # V6 par-stats: s6 is 32-packed (32 blocks x 4 feats); v6' is 64 blocks.
    V6 = []
    for par in range(2):
        S = np.zeros((128, 64), np.float32)
        for m in range(64):
            if m % 2 != par:
                continue
            g = m // 2
            S[g * dout6:(g + 1) * dout6, m] = 1.0 / dout6
        V6.append(S)
    V7 = np.zeros((128, 64), np.float32)
    for m in range(64):
        V7[m * dout7:(m + 1) * dout7, m] = 1.0 / dout7
    return V6[0], V6[1], V7


def _numpy_forward(inp):
    h = np.asarray(inp["x"], np.float32)
    for i in range(7):
        W = np.asarray(inp[f"W{i+1}"], np.float32)
        g = np.asarray(inp[f"g{i+1}"], np.float32)
        b = np.asarray(inp[f"bt{i+1}"], np.float32)
        h = h @ W
        m = h.mean(-1, keepdims=True)
        v = np.square(h - m).mean(-1, keepdims=True)
        h = (h - m) / np.sqrt(v + EPS) * g + b
        h = _lrelu(h)
    return (h @ np.asarray(inp["W8"], np.float32)
            + np.asarray(inp["b8"], np.float32)).astype(np.float32)


def _build_consts(inp):
    """Host-side weight prep (gamma folded into next W). Returns fp16 pack."""
    gs = [np.asarray(inp[f"g{l}"], np.float64) for l in range(1, 8)]
    Ws = [np.asarray(inp[f"W{l}"], np.float64) for l in range(1, 8)]
    # fold gamma_{l-1} into W_l rows; gamma_7 into W8
    Wf = [Ws[0]]
    for i in range(1, 7):
        Wf.append(np.diag(gs[i - 1]) @ Ws[i])
    W8f = (np.diag(gs[6]) @ np.asarray(inp["W8"], np.float64)).astype(np.float32)
    Wc = [_center(Wf[i]).astype(np.float32) for i in range(7)]

    cols = {}
    def add(name, arr32):
        hi, lo = _split(arr32)
        cols[name + "h"], cols[name + "l"] = hi, lo

    add("s1", Wc[0])                                   # [128, 32]
    # L2 row-tiled: blockdiag2(Wc2) [64,128] stacked twice -> [128,128]
    bd2 = _blockdiag(Wc[1], 2)
    add("s2", np.vstack([bd2, bd2]))
    add("s3", _blockdiag(Wc[2], 2))                    # [128, 64]
    for l, c_in in ((4, 4), (5, 8), (6, 16), (7, 32)):
        t0, t1 = _transition_stat(Wc[l - 1], c_in)
        add(f"t{l}a", t0)
        add(f"t{l}b", t1)
    add("s8", _blockdiag(W8f, 64))                     # [128, 64]

    # pack all fp16 stationaries into one [128, T] array; remember offsets
    order = sorted(cols.keys())
    offs, total = {}, 0
    for k in order:
        offs[k] = total
        total += cols[k].shape[1]
    wpack = np.zeros((128, total), F16)
    for k in order:
        wpack[:, offs[k]:offs[k] + cols[k].shape[1]] = cols[k]

    V6a, V6b, V7 = _var_stats(DIMS[6], DIMS[7])
    vpack = np.concatenate([V6a, V6b, V7], axis=1).astype(np.float32)
    return wpack, offs, vpack


def _split_multi_waits(nc):
    """Walrus build limit: <=1 sync wait per instruction. Hoist extras onto
    same-engine NOPs inserted just before the instruction."""
    import concourse.mybir as mybir
    import bass_rust
    cnt = 0
    for f in nc.m.functions:
        for blk in f.blocks:
            newlist = []
            for inst in blk.instructions:
                si = inst.sync_info
                waits = list(si.on_wait) if si is not None and si.on_wait else []
                if len(waits) > 1:
                    for w in waits[:-1]:
                        nop = mybir.InstNoOp(name=f"waitnop_{cnt}", ins=[], outs=[])
                        cnt += 1
                        nop.engine = inst.engine
                        nop.sync_info = bass_rust.SyncInfo(on_wait=[w], on_update=[])
                        newlist.append(nop)
                    inst.sync_info = bass_rust.SyncInfo(
                        on_wait=[waits[-1]], on_update=list(si.on_update))
                newlist.append(inst)
            blk.instructions = newlist
    return cnt


def _build_program(offs, wpack_cols, b8_val):
    import concourse.bass as bass
    import concourse.mybir as mybir
    from concourse.tile import TileContext
    from contextlib import ExitStack

    # this walrus build rejects >1 sync wait on the tail Drain; split them
    import bass_rust
    from concourse.tile import TileContext as _TC
    from concourse.vector_clock import ScopedClock

    def _patched_drain(self, tick_clock, wait_clock):
        probe = self.nc.sync.nop()
        wait_clock.add_sem_waits(probe.ins,
                                 ScopedClock({None: tick_clock.global_clock}))
        si = probe.ins.sync_info
        waits = list(si.on_wait) if si is not None else []
        upd = list(si.on_update) if si is not None else []
        probe.ins.sync_info = bass_rust.SyncInfo(on_wait=waits[:1], on_update=upd)
        for w in waits[1:]:
            nop = self.nc.sync.nop()
            nop.ins.sync_info = bass_rust.SyncInfo(on_wait=[w], on_update=[])
        self.nc.sync.drain()
        self.nc.all_engine_barrier()
        assert self.sems is not None
        popped = self.nc._tile_sem_poison_stack.pop()
        assert popped is self._sem_poison
        self.nc.clear_and_free_semaphores(list(self.sems.allocated().values()))
        self.nc.all_engine_barrier()

    _TC._drain_and_barrier = _patched_drain

    f16, f32 = mybir.dt.float16, mybir.dt.float32
    AF = mybir.ActivationFunctionType
    OP = mybir.AluOpType

    nc = bass.Bass(trn_type="TRN2", num_swdge_queues=4)
    xhi_d = nc.dram_tensor("xhi", [128, RPC], f16, kind="ExternalInput")
    xlo_d = nc.dram_tensor("xlo", [128, RPC], f16, kind="ExternalInput")
    wp_d = nc.dram_tensor("wpack", [128, wpack_cols], f16, kind="ExternalInput")
    vp_d = nc.dram_tensor("vpack", [128, 192], f32, kind="ExternalInput")
    out_d = nc.dram_tensor("out", [N_ST * 64, R_ST // 64], f32,
                           kind="ExternalOutput")

    with TileContext(nc) as tc:
        with ExitStack() as ctx:
            const = ctx.enter_context(tc.tile_pool(name="const", bufs=1))
            wp = const.tile([128, wpack_cols], f16)
            nc.sync.dma_start(wp[:, :], wp_d[:, :])
            vpk = const.tile([128, 192], f32)
            nc.sync.dma_start(vpk[:, :], vp_d[:, :])

            def W(name):
                return wp[:, offs[name]:offs[name] + _WCOLS[name]]

            xp = ctx.enter_context(tc.tile_pool(name="xp", bufs=2))
            ap = ctx.enter_context(tc.tile_pool(name="ap", bufs=2))
            fin = ctx.enter_context(tc.tile_pool(name="fin", bufs=N_ST))
            up = ctx.enter_context(tc.tile_pool(name="up", bufs=3, space="PSUM"))
            vp = ctx.enter_context(tc.tile_pool(name="vp", bufs=2, space="PSUM"))

            def mm(out, lhsT, rhs, start, stop, tp=None):
                # matmul output must fit one PSUM bank: 512 fp32 columns
                n = out.shape[1]
                for o in range(0, n, 512):
                    e = min(o + 512, n)
                    nc.tensor.matmul(out[:, o:e], lhsT, rhs[:, o:e],
                                     start=start, stop=stop, tile_position=tp)

            ysbs, e8sbs = [], []
            dma_engs = [nc.sync, nc.gpsimd, nc.scalar, nc.gpsimd]
            xtiles = {}

            def emit_dma(st):
                x0 = st * R_ST
                xh, xl = [], []
                for k in range(2):
                    xht = xp.tile([128, 4096], f16, name=f"xh{k}")
                    dma_engs[k].dma_start(
                        xht[:, :],
                        xhi_d[:, x0 + 4096 * k:x0 + 4096 * (k + 1)])
                    xh.append(xht)
                    xlt = xp.tile([128, 4096], f16, name=f"xl{k}")
                    dma_engs[2 + k].dma_start(
                        xlt[:, :],
                        xlo_d[:, x0 + 4096 * k:x0 + 4096 * (k + 1)])
                    xl.append(xlt)
                xtiles[st] = (xh, xl)

            # per-layer engine choice for the cast (B) and residual (C) pass
            def _b_eng(l):
                return nc.vector if l in (1, 2) else nc.gpsimd

            def _c_eng(l):
                return nc.vector if l in (1, 2, 3) else nc.gpsimd

            def act_split(l, u, n, hi, lo, col0):
                af = ap.tile([128, 1024], f32, name="af", tag="af", bufs=4)
                afv = af[:, :n]
                nc.scalar.activation(afv, u[:, :n], AF.Prelu,
                                     bias=0.0, scale=1.0, alpha=SLOPE)
                _b_eng(l).tensor_copy(hi[:, col0:col0 + n], afv)
                _c_eng(l).tensor_tensor(lo[:, col0:col0 + n], afv,
                                        hi[:, col0:col0 + n], OP.subtract)

            SA = {}  # per-supertile phase-A state
            SB = {}  # per-supertile phase-B (ladder) state

            s2hs = [wp[:, offs["s2h"]:offs["s2h"] + 128][64 * q:64 * (q + 1), :]
                    for q in range(2)]
            s2ls = [wp[:, offs["s2l"]:offs["s2l"] + 128][64 * q:64 * (q + 1), :]
                    for q in range(2)]

            def emit_A(st, i):
                s = SA.setdefault(st, {})
                if i == 0:
                    s["a1h"] = ap.tile([128, 2048], f16, name="a1h")
                    s["a1l"] = ap.tile([128, 2048], f16, name="a1l")
                if i in (0, 1):
                    # ---- L1 chunk c=i: col-tiled 4x [128,32] at (0,32b)
                    c = i
                    xh, xl = xtiles[st]
                    u = up.tile([128, 1024], f32, name="u", tag="u")
                    for t in range(3):
                        for b in range(4):
                            rh = xh[b // 2][:, (b % 2) * 2048 + 1024 * c:][:, :1024]
                            rl = xl[b // 2][:, (b % 2) * 2048 + 1024 * c:][:, :1024]
                            S = W("s1h") if t < 2 else W("s1l")
                            r = rh if t != 1 else rl
                            mm(u[32 * b:32 * (b + 1), :], S, r,
                               start=(t == 0), stop=(t == 2), tp=(0, 32 * b))
                    act_split(1, u, 1024, s["a1h"], s["a1l"], 1024 * c)
                    if i == 1:
                        # all readers of this supertile's x tiles are emitted;
                        # prefetch the st+2 input into the freed xp buffers
                        del xtiles[st]
                        if st + 2 < N_ST:
                            emit_dma(st + 2)
                if i == 2:
                    s["a2h"] = [ap.tile([128, 2048], f16, name=f"a2h{q}")
                                for q in range(2)]
                    s["a2l"] = [ap.tile([128, 2048], f16, name=f"a2l{q}")
                                for q in range(2)]
                if i in (2, 3):
                    # ---- L2 chunk c=i-2: row-tiled 2x [64,128]
                    c = i - 2
                    us = [up.tile([128, 1024], f32, name="u", tag="u")
                          for _ in range(2)]
                    for t in range(3):
                        for q in range(2):
                            rh = s["a1h"][64 * q:64 * (q + 1),
                                          1024 * c:1024 * (c + 1)]
                            rl = s["a1l"][64 * q:64 * (q + 1),
                                          1024 * c:1024 * (c + 1)]
                            S = s2hs[q] if t < 2 else s2ls[q]
                            r = rh if t != 1 else rl
                            mm(us[q][:, :], S, r, start=(t == 0),
                               stop=(t == 2), tp=(64 * q, 0))
                    for q in range(2):
                        act_split(2, us[q], 1024, s["a2h"][q], s["a2l"][q],
                                  1024 * c)
                if i == 4:
                    s["a3h"] = ap.tile([128, 2048], f16, name="a3h")
                    s["a3l"] = ap.tile([128, 2048], f16, name="a3l")
                if i in (4, 5):
                    # ---- L3 chunk c=i-4: col-tiled 2x [128,64]
                    c = i - 4
                    u = up.tile([128, 1024], f32, name="u", tag="u")
                    for t in range(3):
                        for q in range(2):
                            rh = s["a2h"][q][:, 1024 * c:1024 * (c + 1)]
                            rl = s["a2l"][q][:, 1024 * c:1024 * (c + 1)]
                            S = W("s3h") if t < 2 else W("s3l")
                            r = rh if t != 1 else rl
                            mm(u[64 * q:64 * (q + 1), :], S, r,
                               start=(t == 0), stop=(t == 2), tp=(0, 64 * q))
                    act_split(3, u, 1024, s["a3h"], s["a3l"], 1024 * c)

            def emit_B(st, i):
                s = SA[st]
                b = SB.setdefault(st, {})
                if i == 0:
                    b["ph"], b["pl"], b["n"] = s["a3h"], s["a3l"], 2048
                if i < 4:
                    # ---- L4+i: parity transition, halving free size
                    l = 4 + i
                    n = b["n"] // 2
                    u = up.tile([128, 1024], f32, name="u", tag="u")
                    uv = u[:, :n]
                    first = True
                    for par, suf in ((0, "a"), (1, "b")):
                        rh = b["ph"][:, par * n:(par + 1) * n]
                        rl = b["pl"][:, par * n:(par + 1) * n]
                        th, tl = W(f"t{l}{suf}h"), W(f"t{l}{suf}l")
                        mm(uv, th, rh, start=first, stop=False)
                        mm(uv, th, rl, start=False, stop=False)
                        mm(uv, tl, rh, start=False, stop=(par == 1))
                        first = False
                    nh = ap.tile([128, n], f16, name=f"a{l}h")
                    nl = ap.tile([128, n], f16, name=f"a{l}l")
                    if l == 6:
                        # fp32 squares (fp16 would underflow degenerate rows)
                        b["s6"] = ap.tile([128, 256], f32, name="s6")
                        nc.scalar.activation(b["s6"][:, :], uv, AF.Square)
                    if l == 7:
                        b["s7"] = ap.tile([128, 128], f32, name="s7")
                        nc.scalar.activation(b["s7"][:, :], uv, AF.Square)
                    act_split(l, u, n, nh, nl, 0)
                    b["ph"], b["pl"], b["n"] = nh, nl, n
                    return
                # ---- i == 4: tail. L8 first (no deps on variances).
                yt = vp.tile([64, 128], f32, name="yt", tag="v")
                mm(yt[:, :], W("s8h"), b["ph"][:, :], start=True, stop=False)
                mm(yt[:, :], W("s8h"), b["pl"][:, :], start=False, stop=False)
                mm(yt[:, :], W("s8l"), b["ph"][:, :], start=False, stop=True)
                ysb = fin.tile([64, 128], f32, name="ysb", tag="ysb")
                nc.scalar.copy(ysb[:, :], yt[:, :])
                v6t = vp.tile([64, 128], f32, name="v6t", tag="v")
                mm(v6t[:, :], vpk[:, 0:64], b["s6"][:, 0:128],
                   start=True, stop=False)
                mm(v6t[:, :], vpk[:, 64:128], b["s6"][:, 128:256],
                   start=False, stop=True)
                v7t = vp.tile([64, 128], f32, name="v7t", tag="v")
                mm(v7t[:, :], vpk[:, 128:192], b["s7"][:, :],
                   start=True, stop=True)
                v7sb = fin.tile([64, 128], f32, name="v7sb", tag="v7sb", bufs=2)
                nc.scalar.copy(v7sb[:, :], v7t[:, :])
                e8 = fin.tile([64, 128], f32, name="e8", tag="e8")
                nc.vector.scalar_tensor_tensor(e8[:, :], v6t[:, :], EPS,
                                               v7sb[:, :], OP.mult, OP.add)
                ysbs.append(ysb)
                e8sbs.append(e8)
                SA.pop(st)
                SB.pop(st)

            emit_dma(0)
            emit_dma(1)
            for st in range(N_ST + 1):
                for i in range(6):
                    if st < N_ST:
                        emit_A(st, i)
                    if st >= 1 and i < 5:
                        emit_B(st - 1, i)

            # ---- final: out = y / sqrt(E8) + b8 (one Sqrt table-load)
            for st in range(N_ST):
                sq = fin.tile([64, 128], f32, name="sq", tag="sq", bufs=2)
                nc.scalar.activation(sq[:, :], e8sbs[st][:, :], AF.Sqrt)
                rinv = fin.tile([64, 128], f32, name="rinv", tag="rinv", bufs=2)
                nc.vector.reciprocal(rinv[:, :], sq[:, :])
                osb = fin.tile([64, 128], f32, name="osb", tag="osb", bufs=2)
                nc.vector.tensor_tensor(osb[:, :], ysbs[st][:, :], rinv[:, :],
                                        OP.mult)
                nc.vector.tensor_scalar(osb[:, :], osb[:, :], b8_val,
                                        None, OP.add)
                nc.sync.dma_start(out_d[st * 64:(st + 1) * 64, :], osb[:, :])
    _split_multi_waits(nc)
    return nc


_WCOLS = {}


def kernel(**inputs):
    for l in range(1, 8):
        if np.abs(np.asarray(inputs[f"bt{l}"], np.float32)).max() > 0:
            return _numpy_forward(inputs)
        if np.asarray(inputs[f"g{l}"], np.float32).min() <= 0:
            return _numpy_forward(inputs)

    wpack, offs, vpack = _build_consts(inputs)
    global _WCOLS
    _WCOLS = {"s1h": 32, "s1l": 32, "s2h": 128, "s2l": 128,
              "s3h": 64, "s3l": 64, "s8h": 64, "s8l": 64}
    for l in range(4, 8):
        for suf in ("a", "b"):
            _WCOLS[f"t{l}{suf}h"] = 128
            _WCOLS[f"t{l}{suf}l"] = 128

    x = np.asarray(inputs["x"], np.float32)
    xT = np.ascontiguousarray(x.T)               # [128, 524288]
    xhi = xT.astype(F16)
    xlo = (xT - xhi.astype(np.float32)).astype(F16)
    b8 = np.asarray(inputs["b8"], np.float32).reshape(1, 1)

    nc = _build_program(offs, wpack.shape[1], float(b8[0, 0]))

    in_maps = []
    for c in range(N_CORES):
        s = slice(c * RPC, (c + 1) * RPC)
        in_maps.append({
            "xhi": np.ascontiguousarray(xhi[:, s]),
            "xlo": np.ascontiguousarray(xlo[:, s]),
            "wpack": wpack, "vpack": vpack,
        })

    from concourse.bass_utils import run_bass_kernel_spmd
    res = run_bass_kernel_spmd(nc, in_maps, core_ids=list(range(N_CORES)))

    out = np.empty((ROWS, 1), np.float32)
    for c in range(N_CORES):
        out[c * RPC:(c + 1) * RPC, 0] = res.results[c]["out"].reshape(-1)
    return out


# revision 19
# speedup vs baseline: 1.6520x; 1.6334x over previous
"""Trainium2 Bass kernel for nn_DiscriminatorModel (8-layer MLP with
LayerNorm+LeakyReLU, 524288x128 input, data-parallel over 8 NeuronCores).

Algorithm (validated vs the jax reference to ~7e-4 relative absmax):
  - Mean-centering of each LayerNorm is folded into the weights host-side:
    Wc_l = W_l @ (I - 11^T/d)  => matmul output is already centered.
  - LayerNorm gammas are folded into the NEXT layer's weights host-side
    (LReLU(g*z) = g*LReLU(z) for g > 0), so the on-device activation is a
    plain LeakyReLU with no per-feature scale.
  - The per-row rsqrt(var+eps) scales commute through LeakyReLU and the
    following matmul; only the layer-6/7 variances matter to fp32 precision:
        E8 = v7 + eps*v6,   out = (a7 @ W8) / sqrt(E8) + b8
  - fp32-grade precision via fp16 multi-word matmuls (3 terms):
        z = Wh@ah + Wh@al + Wl@ah,  fp32 PSUM accumulate.
  - Activations are packed feature-major: 128 partitions = c blocks x dout
    features, rows along the free dim. Col/row tile_position packing keeps
    concurrent matmuls on the PE array.
  - Software pipelining: the serial L4->L7 ladder of supertile k is emitted
    interleaved with the wide L1-L3 matmuls of supertile k+1, so the PE
    never waits on the act chain; input DMA is prefetched 2 supertiles
    ahead. Per layer the activation split is three elementwise passes:
        A: af32 = LReLU(psum) [ScalarE]
        B: ah = fp16(af32)    [VectorE or GpSimd per layer]
        C: al = af32 - ah     [VectorE or GpSimd per layer]

Requires all LayerNorm beta == 0 and gamma > 0 (true for the reference
inputs); otherwise falls back to a numpy forward pass.
"""

import numpy as np

EPS = 1e-5
SLOPE = 0.2
DIMS = [128, 32, 64, 32, 16, 8, 4, 2]
N_CORES = 8
ROWS = 524288
RPC = ROWS // N_CORES        # 65536 rows per core
R_ST = 8192                  # rows per supertile
N_ST = RPC // R_ST           # 8 supertiles per core
F16 = np.float16

_CACHE = {}


def _lrelu(x):
    return np.where(x > 0, x, SLOPE * x).astype(np.float32)


def _center(W):
    d = W.shape[1]
    return (np.asarray(W, np.float64) @ (np.eye(d) - 1.0 / d))


def _split(a):
    hi = a.astype(F16)
    lo = (a.astype(np.float32) - hi.astype(np.float32)).astype(F16)
    return hi, lo


def _blockdiag(W, c):
    din, dout = W.shape
    out = np.zeros((c * din, c * dout), W.dtype)
    for b in range(c):
        out[b * din:(b + 1) * din, b * dout:(b + 1) * dout] = W
    return out


def _transition_stat(W, c_in):
    """Parity-interleaved stationary for a c_in -> 2*c_in packing transition.

    Two stats (par=0,1), each [128, 128]: out col m = blk_out*w + f where
    w = 128/(2*c_in) per-block output width; nonzero iff blk_out % 2 == par,
    source block g = blk_out // 2 maps rows g*din..(g+1)*din <- W[:, f].
    """
    din, dout = W.shape
    w = 128 // (2 * c_in)
    assert w == dout
    stats = []
    for par in range(2):
        S = np.zeros((128, 128), W.dtype)
        for m in range(128):
            blk_out, f = divmod(m, w)
            if blk_out % 2 != par:
                continue
            g = blk_out // 2
            S[g * din:(g + 1) * din, m] = W[:, f]
        stats.append(S)
    return stats


def _var_stats(dout6, dout7):
    # V6 par-stats: s6 is 32-packed (32 blocks x 4 feats); v6' is 64 blocks.
    V6 = []
    for par in range(2):
        S = np.zeros((128, 64), np.float32)
        for m in range(64):
            if m % 2 != par:
                continue
            g = m // 2
            S[g * dout6:(g + 1) * dout6, m] = 1.0 / dout6
        V6.append(S)
    V7 = np.zeros((128, 64), np.float32)
    for m in range(64):
        V7[m * dout7:(m + 1) * dout7, m] = 1.0 / dout7
    return V6[0], V6[1], V7


def _numpy_forward(inp):
    h = np.asarray(inp["x"], np.float32)
    for i in range(7):
        W = np.asarray(inp[f"W{i+1}"], np.float32)
        g = np.asarray(inp[f"g{i+1}"], np.float32)
        b = np.asarray(inp[f"bt{i+1}"], np.float32)
        h = h @ W
        m = h.mean(-1, keepdims=True)
        v = np.square(h - m).mean(-1, keepdims=True)
        h = (h - m) / np.sqrt(v + EPS) * g + b
        h = _lrelu(h)
    return (h @ np.asarray(inp["W8"], np.float32)
            + np.asarray(inp["b8"], np.float32)).astype(np.float32)


def _build_consts(inp):
    """Host-side weight prep (gamma folded into next W). Returns fp16 pack."""
    gs = [np.asarray(inp[f"g{l}"], np.float64) for l in range(1, 8)]
    Ws = [np.asarray(inp[f"W{l}"], np.float64) for l in range(1, 8)]
    # fold gamma_{l-1} into W_l rows; gamma_7 into W8
    Wf = [Ws[0]]
    for i in range(1, 7):
        Wf.append(np.diag(gs[i - 1]) @ Ws[i])
    W8f = (np.diag(gs[6]) @ np.asarray(inp["W8"], np.float64)).astype(np.float32)
    Wc = [_center(Wf[i]).astype(np.float32) for i in range(7)]

    # L1 runs as 3-term fp16 (x arrives as an fp16 hi/lo pair); every later
    # layer runs a single exact-fp32 matmul, so those stationaries are fp32.
    c16 = {}
    h1, l1 = _split(Wc[0])
    c16["s1h"], c16["s1l"] = h1, l1

    c32 = {}
    bd2 = _blockdiag(Wc[1], 2)
    c32["s2"] = np.vstack([bd2, bd2]).astype(np.float32)
    c32["s3"] = _blockdiag(Wc[2], 2).astype(np.float32)
    for l, c_in in ((4, 4), (5, 8), (6, 16), (7, 32)):
        t0, t1 = _transition_stat(Wc[l - 1], c_in)
        c32[f"t{l}a"] = t0.astype(np.float32)
        c32[f"t{l}b"] = t1.astype(np.float32)
    c32["s8"] = _blockdiag(W8f, 64).astype(np.float32)
    V6a, V6b, V7 = _var_stats(DIMS[6], DIMS[7])
    c32["v6a"], c32["v6b"], c32["v7"] = V6a, V6b, V7

    def pack(cols, dt):
        order = sorted(cols.keys())
        offs, total = {}, 0
        for k in order:
            offs[k] = total
            total += cols[k].shape[1]
        arr = np.zeros((128, total), dt)
        for k in order:
            arr[:, offs[k]:offs[k] + cols[k].shape[1]] = cols[k]
        return arr, offs

    wpack, offs16 = pack(c16, F16)
    wpack32, offs32 = pack(c32, np.float32)
    return wpack, offs16, wpack32, offs32


def _split_multi_waits(nc):
    """Walrus build limit: <=1 sync wait per instruction. Hoist extras onto
    same-engine NOPs inserted just before the instruction."""
    import concourse.mybir as mybir
    import bass_rust
    cnt = 0
    for f in nc.m.functions:
        for blk in f.blocks:
            newlist = []
            for inst in blk.instructions:
                si = inst.sync_info
                waits = list(si.on_wait) if si is not None and si.on_wait else []
                if len(waits) > 1:
                    for w in waits[:-1]:
                        nop = mybir.InstNoOp(name=f"waitnop_{cnt}", ins=[], outs=[])
                        cnt += 1
                        nop.engine = inst.engine
                        nop.sync_info = bass_rust.SyncInfo(on_wait=[w], on_update=[])
                        newlist.append(nop)
                    inst.sync_info = bass_rust.SyncInfo(
                        on_wait=[waits[-1]], on_update=list(si.on_update))
                newlist.append(inst)
            blk.instructions = newlist
    return cnt


def _build_program(offs16, w16_cols, offs32, w32_cols, b8_val):
    import concourse.bass as bass
    import concourse.mybir as mybir
    from concourse.tile import TileContext
    from contextlib import ExitStack

    # this walrus build rejects >1 sync wait on the tail Drain; split them
    import bass_rust
    from concourse.tile import TileContext as _TC
    from concourse.vector_clock import ScopedClock

    def _patched_drain(self, tick_clock, wait_clock):
        probe = self.nc.sync.nop()
        wait_clock.add_sem_waits(probe.ins,
                                 ScopedClock({None: tick_clock.global_clock}))
        si = probe.ins.sync_info
        waits = list(si.on_wait) if si is not None else []
        upd = list(si.on_update) if si is not None else []
        probe.ins.sync_info = bass_rust.SyncInfo(on_wait=waits[:1], on_update=upd)
        for w in waits[1:]:
            nop = self.nc.sync.nop()
            nop.ins.sync_info = bass_rust.SyncInfo(on_wait=[w], on_update=[])
        self.nc.sync.drain()
        self.nc.all_engine_barrier()
        assert self.sems is not None
        popped = self.nc._tile_sem_poison_stack.pop()
        assert popped is self._sem_poison
        self.nc.clear_and_free_semaphores(list(self.sems.allocated().values()))
        self.nc.all_engine_barrier()

    _TC._drain_and_barrier = _patched_drain

    f16, f32 = mybir.dt.float16, mybir.dt.float32
    AF = mybir.ActivationFunctionType
    OP = mybir.AluOpType

    nc = bass.Bass(trn_type="TRN2", num_swdge_queues=4)
    xhi_d = nc.dram_tensor("xhi", [128, RPC], f16, kind="ExternalInput")
    xlo_d = nc.dram_tensor("xlo", [128, RPC], f16, kind="ExternalInput")
    wp_d = nc.dram_tensor("wpack", [128, w16_cols], f16, kind="ExternalInput")
    wp32_d = nc.dram_tensor("wpack32", [128, w32_cols], f32,
                            kind="ExternalInput")
    out_d = nc.dram_tensor("out", [N_ST * 64, R_ST // 64], f32,
                           kind="ExternalOutput")

    with TileContext(nc) as tc:
        with ExitStack() as ctx:
            const = ctx.enter_context(tc.tile_pool(name="const", bufs=1))
            wp = const.tile([128, w16_cols], f16)
            nc.sync.dma_start(wp[:, :], wp_d[:, :])
            wp32 = const.tile([128, w32_cols], f32)
            nc.sync.dma_start(wp32[:, :], wp32_d[:, :])

            def W16(name):
                return wp[:, offs16[name]:offs16[name] + _WCOLS16[name]]

            def W32(name):
                return wp32[:, offs32[name]:offs32[name] + _WCOLS32[name]]

            xp = ctx.enter_context(tc.tile_pool(name="xp", bufs=2))
            ap = ctx.enter_context(tc.tile_pool(name="ap", bufs=2))
            fin = ctx.enter_context(tc.tile_pool(name="fin", bufs=N_ST))
            up = ctx.enter_context(tc.tile_pool(name="up", bufs=3, space="PSUM"))
            vp = ctx.enter_context(tc.tile_pool(name="vp", bufs=2, space="PSUM"))

            def mm(out, lhsT, rhs, start, stop, tp=None):
                # matmul output must fit one PSUM bank: 512 fp32 columns
                n = out.shape[1]
                for o in range(0, n, 512):
                    e = min(o + 512, n)
                    nc.tensor.matmul(out[:, o:e], lhsT, rhs[:, o:e],
                                     start=start, stop=stop, tile_position=tp)

            ysbs, e8sbs = [], []
            dma_engs = [nc.sync, nc.gpsimd, nc.scalar, nc.gpsimd]
            xtiles = {}

            def emit_dma(st):
                x0 = st * R_ST
                xh, xl = [], []
                for k in range(2):
                    xht = xp.tile([128, 4096], f16, name=f"xh{k}")
                    dma_engs[k].dma_start(
                        xht[:, :],
                        xhi_d[:, x0 + 4096 * k:x0 + 4096 * (k + 1)])
                    xh.append(xht)
                    xlt = xp.tile([128, 4096], f16, name=f"xl{k}")
                    dma_engs[2 + k].dma_start(
                        xlt[:, :],
                        xlo_d[:, x0 + 4096 * k:x0 + 4096 * (k + 1)])
                    xl.append(xlt)
                xtiles[st] = (xh, xl)

            def act(u, n, dst, col0, eng=None):
                """Single LeakyReLU pass: PSUM fp32 -> SBUF fp32 act tile."""
                (eng or nc.scalar).activation(
                    dst[:, col0:col0 + n], u[:, :n], AF.Prelu,
                    bias=0.0, scale=1.0, alpha=SLOPE)

            SA = {}  # per-supertile phase-A state
            SB = {}  # per-supertile phase-B (ladder) state

            s2s = [W32("s2")[64 * q:64 * (q + 1), :] for q in range(2)]

            def emit_A(st, i):
                s = SA.setdefault(st, {})
                if i == 0:
                    s["a1"] = ap.tile([128, 2048], f32, name="a1")
                if i in (0, 1):
                    # ---- L1 chunk c=i: fp16 3-term, col-tiled 4x [128,32]
                    c = i
                    xh, xl = xtiles[st]
                    u = up.tile([128, 1024], f32, name="u", tag="u")
                    for t in range(3):
                        for b in range(4):
                            rh = xh[b // 2][:, (b % 2) * 2048 + 1024 * c:][:, :1024]
                            rl = xl[b // 2][:, (b % 2) * 2048 + 1024 * c:][:, :1024]
                            S = W16("s1h") if t < 2 else W16("s1l")
                            r = rh if t != 1 else rl
                            mm(u[32 * b:32 * (b + 1), :], S, r,
                               start=(t == 0), stop=(t == 2), tp=(0, 32 * b))
                    act(u, 1024, s["a1"], 1024 * c)
                    if i == 1:
                        # all readers of this supertile's x tiles are emitted;
                        # prefetch the st+2 input into the freed xp buffers
                        del xtiles[st]
                        if st + 2 < N_ST:
                            emit_dma(st + 2)
                if i == 2:
                    s["a2"] = [ap.tile([128, 2048], f32, name=f"a2q{q}")
                               for q in range(2)]
                if i in (2, 3):
                    # ---- L2 chunk c=i-2: fp32, row-tiled 2x [64,128]
                    c = i - 2
                    us = [up.tile([128, 1024], f32, name="u", tag="u")
                          for _ in range(2)]
                    for q in range(2):
                        mm(us[q][:, :], s2s[q],
                           s["a1"][64 * q:64 * (q + 1), 1024 * c:1024 * (c + 1)],
                           start=True, stop=True, tp=(64 * q, 0))
                    for q in range(2):
                        act(us[q], 1024, s["a2"][q], 1024 * c)
                if i == 4:
                    s["a3"] = ap.tile([128, 2048], f32, name="a3")
                if i in (4, 5):
                    # ---- L3 chunk c=i-4: fp32, col-tiled 2x [128,64]
                    c = i - 4
                    u = up.tile([128, 1024], f32, name="u", tag="u")
                    for q in range(2):
                        mm(u[64 * q:64 * (q + 1), :], W32("s3"),
                           s["a2"][q][:, 1024 * c:1024 * (c + 1)],
                           start=True, stop=True, tp=(0, 64 * q))
                    act(u, 1024, s["a3"], 1024 * c)

            def emit_B(st, i):
                s = SA[st]
                b = SB.setdefault(st, {})
                if i == 0:
                    b["prev"], b["n"] = s["a3"], 2048
                if i < 4:
                    # ---- L4+i: fp32 parity transition, halving free size
                    l = 4 + i
                    n = b["n"] // 2
                    u = up.tile([128, 1024], f32, name="u", tag="u")
                    uv = u[:, :n]
                    for par, suf in ((0, "a"), (1, "b")):
                        mm(uv, W32(f"t{l}{suf}"),
                           b["prev"][:, par * n:(par + 1) * n],
                           start=(par == 0), stop=(par == 1))
                    na = ap.tile([128, n], f32, name=f"a{l}")
                    if l == 6:
                        # fp32 squares (fp16 would underflow degenerate rows)
                        b["s6"] = ap.tile([128, 256], f32, name="s6")
                        nc.scalar.activation(b["s6"][:, :], uv, AF.Square)
                    if l == 7:
                        b["s7"] = ap.tile([128, 128], f32, name="s7")
                        nc.scalar.activation(b["s7"][:, :], uv, AF.Square)
                    act(u, n, na, 0)
                    b["prev"], b["n"] = na, n
                    return
                # ---- i == 4: tail. L8 first (no deps on variances).
                yt = vp.tile([64, 128], f32, name="yt", tag="v")
                mm(yt[:, :], W32("s8"), b["prev"][:, :], start=True, stop=True)
                ysb = fin.tile([64, 128], f32, name="ysb", tag="ysb")
                nc.scalar.copy(ysb[:, :], yt[:, :])
                v6t = vp.tile([64, 128], f32, name="v6t", tag="v")
                mm(v6t[:, :], W32("v6a"), b["s6"][:, 0:128],
                   start=True, stop=False)
                mm(v6t[:, :], W32("v6b"), b["s6"][:, 128:256],
                   start=False, stop=True)
                v7t = vp.tile([64, 128], f32, name="v7t", tag="v")
                mm(v7t[:, :], W32("v7"), b["s7"][:, :],
                   start=True, stop=True)
                v7sb = fin.tile([64, 128], f32, name="v7sb", tag="v7sb", bufs=2)
                nc.scalar.copy(v7sb[:, :], v7t[:, :])
                e8 = fin.tile([64, 128], f32, name="e8", tag="e8")
                nc.vector.scalar_tensor_tensor(e8[:, :], v6t[:, :], EPS,
                                               v7sb[:, :], OP.mult, OP.add)
                ysbs.append(ysb)
                e8sbs.append(e8)
                SA.pop(st)
                SB.pop(st)

            emit_dma(0)
            emit_dma(1)
            for st in range(N_ST + 1):
                for i in range(6):
                    if st < N_ST:
                        emit_A(st, i)
                    if st >= 1 and i < 5:
                        emit_B(st - 1, i)

            # ---- final: out = y / sqrt(E8) + b8 (one Sqrt table-load)
            for st in range(N_ST):
                sq = fin.tile([64, 128], f32, name="sq", tag="sq", bufs=2)
                nc.scalar.activation(sq[:, :], e8sbs[st][:, :], AF.Sqrt)
                rinv = fin.tile([64, 128], f32, name="rinv", tag="rinv", bufs=2)
                nc.vector.reciprocal(rinv[:, :], sq[:, :])
                osb = fin.tile([64, 128], f32, name="osb", tag="osb", bufs=2)
                nc.vector.tensor_tensor(osb[:, :], ysbs[st][:, :], rinv[:, :],
                                        OP.mult)
                nc.vector.tensor_scalar(osb[:, :], osb[:, :], b8_val,
                                        None, OP.add)
                nc.sync.dma_start(out_d[st * 64:(st + 1) * 64, :], osb[:, :])
    _split_multi_waits(nc)
    return nc


_WCOLS16 = {}
_WCOLS32 = {}


def kernel(**inputs):
    for l in range(1, 8):
        if np.abs(np.asarray(inputs[f"bt{l}"], np.float32)).max() > 0:
            return _numpy_forward(inputs)
        if np.asarray(inputs[f"g{l}"], np.float32).min() <= 0:
            return _numpy_forward(inputs)

    wpack, offs16, wpack32, offs32 = _build_consts(inputs)
    global _WCOLS16, _WCOLS32
    _WCOLS16 = {"s1h": 32, "s1l": 32}
    _WCOLS32 = {"s2": 128, "s3": 64, "s8": 64,
                "v6a": 64, "v6b": 64, "v7": 64}
    for l in range(4, 8):
        for suf in ("a", "b"):
            _WCOLS32[f"t{l}{suf}"] = 128

    x = np.asarray(inputs["x"], np.float32)
    xT = np.ascontiguousarray(x.T)               # [128, 524288]
    xhi = xT.astype(F16)
    xlo = (xT - xhi.astype(np.float32)).astype(F16)
    b8 = np.asarray(inputs["b8"], np.float32).reshape(1, 1)

    nc = _build_program(offs16, wpack.shape[1], offs32, wpack32.shape[1],
                        float(b8[0, 0]))

    in_maps = []
    for c in range(N_CORES):
        s = slice(c * RPC, (c + 1) * RPC)
        in_maps.append({
            "xhi": np.ascontiguousarray(xhi[:, s]),
            "xlo": np.ascontiguousarray(xlo[:, s]),
            "wpack": wpack, "wpack32": wpack32,
        })

    from concourse.bass_utils import run_bass_kernel_spmd
    res = run_bass_kernel_spmd(nc, in_maps, core_ids=list(range(N_CORES)))

    out = np.empty((ROWS, 1), np.float32)
    for c in range(N_CORES):
        out[c * RPC:(c + 1) * RPC, 0] = res.results[c]["out"].reshape(-1)
    return out


# revision 21
# speedup vs baseline: 1.6810x; 1.0175x over previous
"""Trainium2 Bass kernel for nn_DiscriminatorModel (8-layer MLP with
LayerNorm+LeakyReLU, 524288x128 input, data-parallel over 8 NeuronCores).

Algorithm (validated vs the jax reference to ~7e-4 relative absmax):
  - Mean-centering of each LayerNorm is folded into the weights host-side:
    Wc_l = W_l @ (I - 11^T/d)  => matmul output is already centered.
  - LayerNorm gammas are folded into the NEXT layer's weights host-side
    (LReLU(g*z) = g*LReLU(z) for g > 0), so the on-device activation is a
    plain LeakyReLU with no per-feature scale.
  - The per-row rsqrt(var+eps) scales commute through LeakyReLU and the
    following matmul; only the layer-6/7 variances matter to fp32 precision:
        E8 = v7 + eps*v6,   out = (a7 @ W8) / sqrt(E8) + b8
  - fp32-grade precision via fp16 multi-word matmuls (3 terms):
        z = Wh@ah + Wh@al + Wl@ah,  fp32 PSUM accumulate.
  - Activations are packed feature-major: 128 partitions = c blocks x dout
    features, rows along the free dim. Col/row tile_position packing keeps
    concurrent matmuls on the PE array.
  - Software pipelining: the serial L4->L7 ladder of supertile k is emitted
    interleaved with the wide L1-L3 matmuls of supertile k+1, so the PE
    never waits on the act chain; input DMA is prefetched 2 supertiles
    ahead. Per layer the activation split is three elementwise passes:
        A: af32 = LReLU(psum) [ScalarE]
        B: ah = fp16(af32)    [VectorE or GpSimd per layer]
        C: al = af32 - ah     [VectorE or GpSimd per layer]

Requires all LayerNorm beta == 0 and gamma > 0 (true for the reference
inputs); otherwise falls back to a numpy forward pass.
"""

import numpy as np

EPS = 1e-5
SLOPE = 0.2
DIMS = [128, 32, 64, 32, 16, 8, 4, 2]
N_CORES = 8
ROWS = 524288
RPC = ROWS // N_CORES        # 65536 rows per core
R_ST = 8192                  # rows per supertile
N_ST = RPC // R_ST           # 8 supertiles per core
F16 = np.float16

_CACHE = {}


def _lrelu(x):
    return np.where(x > 0, x, SLOPE * x).astype(np.float32)


def _center(W):
    d = W.shape[1]
    return (np.asarray(W, np.float64) @ (np.eye(d) - 1.0 / d))


def _split(a):
    hi = a.astype(F16)
    lo = (a.astype(np.float32) - hi.astype(np.float32)).astype(F16)
    return hi, lo


def _blockdiag(W, c):
    din, dout = W.shape
    out = np.zeros((c * din, c * dout), W.dtype)
    for b in range(c):
        out[b * din:(b + 1) * din, b * dout:(b + 1) * dout] = W
    return out


def _transition_stat(W, c_in):
    """Parity-interleaved stationary for a c_in -> 2*c_in packing transition.

    Two stats (par=0,1), each [128, 128]: out col m = blk_out*w + f where
    w = 128/(2*c_in) per-block output width; nonzero iff blk_out % 2 == par,
    source block g = blk_out // 2 maps rows g*din..(g+1)*din <- W[:, f].
    """
    din, dout = W.shape
    w = 128 // (2 * c_in)
    assert w == dout
    stats = []
    for par in range(2):
        S = np.zeros((128, 128), W.dtype)
        for m in range(128):
            blk_out, f = divmod(m, w)
            if blk_out % 2 != par:
                continue
            g = blk_out // 2
            S[g * din:(g + 1) * din, m] = W[:, f]
        stats.append(S)
    return stats


def _var_stats(dout6, dout7):
    # V6 par-stats: s6 is 32-packed (32 blocks x 4 feats); v6' is 64 blocks.
    V6 = []
    for par in range(2):
        S = np.zeros((128, 64), np.float32)
        for m in range(64):
            if m % 2 != par:
                continue
            g = m // 2
            S[g * dout6:(g + 1) * dout6, m] = 1.0 / dout6
        V6.append(S)
    V7 = np.zeros((128, 64), np.float32)
    for m in range(64):
        V7[m * dout7:(m + 1) * dout7, m] = 1.0 / dout7
    return V6[0], V6[1], V7


def _numpy_forward(inp):
    h = np.asarray(inp["x"], np.float32)
    for i in range(7):
        W = np.asarray(inp[f"W{i+1}"], np.float32)
        g = np.asarray(inp[f"g{i+1}"], np.float32)
        b = np.asarray(inp[f"bt{i+1}"], np.float32)
        h = h @ W
        m = h.mean(-1, keepdims=True)
        v = np.square(h - m).mean(-1, keepdims=True)
        h = (h - m) / np.sqrt(v + EPS) * g + b
        h = _lrelu(h)
    return (h @ np.asarray(inp["W8"], np.float32)
            + np.asarray(inp["b8"], np.float32)).astype(np.float32)


def _build_consts(inp):
    """Host-side weight prep (gamma folded into next W). Returns fp16 pack."""
    gs = [np.asarray(inp[f"g{l}"], np.float64) for l in range(1, 8)]
    Ws = [np.asarray(inp[f"W{l}"], np.float64) for l in range(1, 8)]
    # fold gamma_{l-1} into W_l rows; gamma_7 into W8
    Wf = [Ws[0]]
    for i in range(1, 7):
        Wf.append(np.diag(gs[i - 1]) @ Ws[i])
    W8f = (np.diag(gs[6]) @ np.asarray(inp["W8"], np.float64)).astype(np.float32)
    Wc = [_center(Wf[i]).astype(np.float32) for i in range(7)]

    # L1 runs as 3-term fp16 (x arrives as an fp16 hi/lo pair); every later
    # layer runs a single exact-fp32 matmul, so those stationaries are fp32.
    c16 = {}
    h1, l1 = _split(Wc[0])
    c16["s1h"], c16["s1l"] = h1, l1

    c32 = {}
    bd2 = _blockdiag(Wc[1], 2)
    c32["s2"] = np.vstack([bd2, bd2]).astype(np.float32)
    c32["s3"] = _blockdiag(Wc[2], 2).astype(np.float32)
    for l, c_in in ((4, 4), (5, 8), (6, 16), (7, 32)):
        t0, t1 = _transition_stat(Wc[l - 1], c_in)
        c32[f"t{l}a"] = t0.astype(np.float32)
        c32[f"t{l}b"] = t1.astype(np.float32)
    c32["s8"] = _blockdiag(W8f, 64).astype(np.float32)
    V6a, V6b, V7 = _var_stats(DIMS[6], DIMS[7])
    c32["v6a"], c32["v6b"], c32["v7"] = V6a, V6b, V7

    def pack(cols, dt):
        order = sorted(cols.keys())
        offs, total = {}, 0
        for k in order:
            offs[k] = total
            total += cols[k].shape[1]
        arr = np.zeros((128, total), dt)
        for k in order:
            arr[:, offs[k]:offs[k] + cols[k].shape[1]] = cols[k]
        return arr, offs

    wpack, offs16 = pack(c16, F16)
    wpack32, offs32 = pack(c32, np.float32)
    return wpack, offs16, wpack32, offs32


def _split_multi_waits(nc):
    """Walrus build limit: <=1 sync wait per instruction. Hoist extras onto
    same-engine NOPs inserted just before the instruction."""
    import concourse.mybir as mybir
    import bass_rust
    cnt = 0
    for f in nc.m.functions:
        for blk in f.blocks:
            newlist = []
            for inst in blk.instructions:
                si = inst.sync_info
                waits = list(si.on_wait) if si is not None and si.on_wait else []
                if len(waits) > 1:
                    for w in waits[:-1]:
                        nop = mybir.InstNoOp(name=f"waitnop_{cnt}", ins=[], outs=[])
                        cnt += 1
                        nop.engine = inst.engine
                        nop.sync_info = bass_rust.SyncInfo(on_wait=[w], on_update=[])
                        newlist.append(nop)
                    inst.sync_info = bass_rust.SyncInfo(
                        on_wait=[waits[-1]], on_update=list(si.on_update))
                newlist.append(inst)
            blk.instructions = newlist
    return cnt


def _build_program(offs16, w16_cols, offs32, w32_cols, b8_val):
    import concourse.bass as bass
    import concourse.mybir as mybir
    from concourse.tile import TileContext
    from contextlib import ExitStack

    # this walrus build rejects >1 sync wait on the tail Drain; split them
    import bass_rust
    from concourse.tile import TileContext as _TC
    from concourse.vector_clock import ScopedClock

    def _patched_drain(self, tick_clock, wait_clock):
        probe = self.nc.sync.nop()
        wait_clock.add_sem_waits(probe.ins,
                                 ScopedClock({None: tick_clock.global_clock}))
        si = probe.ins.sync_info
        waits = list(si.on_wait) if si is not None else []
        upd = list(si.on_update) if si is not None else []
        probe.ins.sync_info = bass_rust.SyncInfo(on_wait=waits[:1], on_update=upd)
        for w in waits[1:]:
            nop = self.nc.sync.nop()
            nop.ins.sync_info = bass_rust.SyncInfo(on_wait=[w], on_update=[])
        self.nc.sync.drain()
        self.nc.all_engine_barrier()
        assert self.sems is not None
        popped = self.nc._tile_sem_poison_stack.pop()
        assert popped is self._sem_poison
        self.nc.clear_and_free_semaphores(list(self.sems.allocated().values()))
        self.nc.all_engine_barrier()

    _TC._drain_and_barrier = _patched_drain

    f16, f32 = mybir.dt.float16, mybir.dt.float32
    AF = mybir.ActivationFunctionType
    OP = mybir.AluOpType

    nc = bass.Bass(trn_type="TRN2", num_swdge_queues=4)
    xhi_d = nc.dram_tensor("xhi", [128, RPC], f16, kind="ExternalInput")
    xlo_d = nc.dram_tensor("xlo", [128, RPC], f16, kind="ExternalInput")
    wp_d = nc.dram_tensor("wpack", [128, w16_cols], f16, kind="ExternalInput")
    wp32_d = nc.dram_tensor("wpack32", [128, w32_cols], f32,
                            kind="ExternalInput")
    out_d = nc.dram_tensor("out", [N_ST * 64, R_ST // 64], f32,
                           kind="ExternalOutput")

    with TileContext(nc) as tc:
        with ExitStack() as ctx:
            const = ctx.enter_context(tc.tile_pool(name="const", bufs=1))
            wp = const.tile([128, w16_cols], f16)
            nc.sync.dma_start(wp[:, :], wp_d[:, :])
            wp32 = const.tile([128, w32_cols], f32)
            nc.sync.dma_start(wp32[:, :], wp32_d[:, :])

            def W16(name):
                return wp[:, offs16[name]:offs16[name] + _WCOLS16[name]]

            def W32(name):
                return wp32[:, offs32[name]:offs32[name] + _WCOLS32[name]]

            xp = ctx.enter_context(tc.tile_pool(name="xp", bufs=2))
            ap = ctx.enter_context(tc.tile_pool(name="ap", bufs=2))
            fin = ctx.enter_context(tc.tile_pool(name="fin", bufs=N_ST))
            up = ctx.enter_context(tc.tile_pool(name="up", bufs=3, space="PSUM"))
            vp = ctx.enter_context(tc.tile_pool(name="vp", bufs=2, space="PSUM"))

            def mm(out, lhsT, rhs, start, stop, tp=None):
                # matmul output must fit one PSUM bank: 512 fp32 columns
                n = out.shape[1]
                for o in range(0, n, 512):
                    e = min(o + 512, n)
                    nc.tensor.matmul(out[:, o:e], lhsT, rhs[:, o:e],
                                     start=start, stop=stop, tile_position=tp)

            def mm_multi(parts, start, stop):
                """Emit chunk-outer / tile-inner so adjacent instructions hit
                disjoint PE subarrays (avoids FIFO head-of-line blocking)."""
                n = parts[0][0].shape[1]
                for o in range(0, n, 512):
                    e = min(o + 512, n)
                    for out, lhsT, rhs, tp in parts:
                        nc.tensor.matmul(out[:, o:e], lhsT, rhs[:, o:e],
                                         start=start, stop=stop,
                                         tile_position=tp)

            ysbs, e8sbs = [], []
            dma_engs = [nc.sync, nc.gpsimd, nc.scalar, nc.gpsimd]
            xtiles = {}

            def emit_dma(st):
                x0 = st * R_ST
                xh, xl = [], []
                for k in range(2):
                    xht = xp.tile([128, 4096], f16, name=f"xh{k}")
                    dma_engs[k].dma_start(
                        xht[:, :],
                        xhi_d[:, x0 + 4096 * k:x0 + 4096 * (k + 1)])
                    xh.append(xht)
                    xlt = xp.tile([128, 4096], f16, name=f"xl{k}")
                    dma_engs[2 + k].dma_start(
                        xlt[:, :],
                        xlo_d[:, x0 + 4096 * k:x0 + 4096 * (k + 1)])
                    xl.append(xlt)
                xtiles[st] = (xh, xl)

            def act(u, n, dst, col0, eng=None):
                """Single LeakyReLU pass: PSUM fp32 -> SBUF fp32 act tile."""
                (eng or nc.scalar).activation(
                    dst[:, col0:col0 + n], u[:, :n], AF.Prelu,
                    bias=0.0, scale=1.0, alpha=SLOPE)

            SA = {}  # per-supertile phase-A state
            SB = {}  # per-supertile phase-B (ladder) state

            s2s = [W32("s2")[64 * q:64 * (q + 1), :] for q in range(2)]

            def emit_A(st, i):
                s = SA.setdefault(st, {})
                if i == 0:
                    s["a1"] = ap.tile([128, 2048], f32, name="a1")
                if i in (0, 1):
                    # ---- L1 chunk c=i: fp16 3-term, col-tiled 4x [128,32]
                    c = i
                    xh, xl = xtiles[st]
                    u = up.tile([128, 1024], f32, name="u", tag="u")
                    for t in range(3):
                        S = W16("s1h") if t < 2 else W16("s1l")
                        parts = []
                        for b in range(4):
                            xsrc = xh if t != 1 else xl
                            r = xsrc[b // 2][:, (b % 2) * 2048 + 1024 * c:][:, :1024]
                            parts.append((u[32 * b:32 * (b + 1), :], S, r,
                                          (0, 32 * b)))
                        mm_multi(parts, start=(t == 0), stop=(t == 2))
                    act(u, 1024, s["a1"], 1024 * c)
                    if i == 1:
                        # all readers of this supertile's x tiles are emitted;
                        # prefetch the st+2 input into the freed xp buffers
                        del xtiles[st]
                        if st + 2 < N_ST:
                            emit_dma(st + 2)
                if i == 2:
                    s["a2"] = [ap.tile([128, 2048], f32, name=f"a2q{q}")
                               for q in range(2)]
                if i in (2, 3):
                    # ---- L2 chunk c=i-2: fp32, row-tiled 2x [64,128]
                    c = i - 2
                    us = [up.tile([128, 1024], f32, name="u", tag="u")
                          for _ in range(2)]
                    mm_multi([(us[q][:, :], s2s[q],
                               s["a1"][64 * q:64 * (q + 1),
                                       1024 * c:1024 * (c + 1)],
                               (64 * q, 0)) for q in range(2)],
                             start=True, stop=True)
                    for q in range(2):
                        act(us[q], 1024, s["a2"][q], 1024 * c)
                if i == 4:
                    s["a3"] = ap.tile([128, 2048], f32, name="a3")
                if i in (4, 5):
                    # ---- L3 chunk c=i-4: fp32, col-tiled 2x [128,64]
                    c = i - 4
                    u = up.tile([128, 1024], f32, name="u", tag="u")
                    mm_multi([(u[64 * q:64 * (q + 1), :], W32("s3"),
                               s["a2"][q][:, 1024 * c:1024 * (c + 1)],
                               (0, 64 * q)) for q in range(2)],
                             start=True, stop=True)
                    act(u, 1024, s["a3"], 1024 * c)

            def emit_B(st, i):
                s = SA[st]
                b = SB.setdefault(st, {})
                if i == 0:
                    b["prev"], b["n"] = s["a3"], 2048
                if i < 4:
                    # ---- L4+i: fp32 parity transition, halving free size
                    l = 4 + i
                    n = b["n"] // 2
                    u = up.tile([128, 1024], f32, name="u", tag="u")
                    uv = u[:, :n]
                    for par, suf in ((0, "a"), (1, "b")):
                        mm(uv, W32(f"t{l}{suf}"),
                           b["prev"][:, par * n:(par + 1) * n],
                           start=(par == 0), stop=(par == 1))
                    na = ap.tile([128, n], f32, name=f"a{l}")
                    if l == 6:
                        # fp32 squares (fp16 would underflow degenerate rows)
                        b["s6"] = ap.tile([128, 256], f32, name="s6")
                        nc.scalar.activation(b["s6"][:, :], uv, AF.Square)
                    if l == 7:
                        b["s7"] = ap.tile([128, 128], f32, name="s7")
                        nc.scalar.activation(b["s7"][:, :], uv, AF.Square)
                    act(u, n, na, 0)
                    b["prev"], b["n"] = na, n
                    return
                # ---- i == 4: tail. L8 first (no deps on variances).
                yt = vp.tile([64, 128], f32, name="yt", tag="v")
                mm(yt[:, :], W32("s8"), b["prev"][:, :], start=True, stop=True)
                ysb = fin.tile([64, 128], f32, name="ysb", tag="ysb")
                nc.scalar.copy(ysb[:, :], yt[:, :])
                v6t = vp.tile([64, 128], f32, name="v6t", tag="v")
                mm(v6t[:, :], W32("v6a"), b["s6"][:, 0:128],
                   start=True, stop=False)
                mm(v6t[:, :], W32("v6b"), b["s6"][:, 128:256],
                   start=False, stop=True)
                v7t = vp.tile([64, 128], f32, name="v7t", tag="v")
                mm(v7t[:, :], W32("v7"), b["s7"][:, :],
                   start=True, stop=True)
                v7sb = fin.tile([64, 128], f32, name="v7sb", tag="v7sb", bufs=2)
                nc.scalar.copy(v7sb[:, :], v7t[:, :])
                e8 = fin.tile([64, 128], f32, name="e8", tag="e8")
                nc.vector.scalar_tensor_tensor(e8[:, :], v6t[:, :], EPS,
                                               v7sb[:, :], OP.mult, OP.add)
                ysbs.append(ysb)
                e8sbs.append(e8)
                SA.pop(st)
                SB.pop(st)

            emit_dma(0)
            emit_dma(1)
            for st in range(N_ST + 1):
                for i in range(6):
                    if st < N_ST:
                        emit_A(st, i)
                    if st >= 1 and i < 5:
                        emit_B(st - 1, i)

            # ---- final: out = y / sqrt(E8) + b8 (one Sqrt table-load)
            for st in range(N_ST):
                sq = fin.tile([64, 128], f32, name="sq", tag="sq", bufs=2)
                nc.scalar.activation(sq[:, :], e8sbs[st][:, :], AF.Sqrt)
                rinv = fin.tile([64, 128], f32, name="rinv", tag="rinv", bufs=2)
                nc.vector.reciprocal(rinv[:, :], sq[:, :])
                osb = fin.tile([64, 128], f32, name="osb", tag="osb", bufs=2)
                nc.vector.tensor_tensor(osb[:, :], ysbs[st][:, :], rinv[:, :],
                                        OP.mult)
                nc.vector.tensor_scalar(osb[:, :], osb[:, :], b8_val,
                                        None, OP.add)
                nc.sync.dma_start(out_d[st * 64:(st + 1) * 64, :], osb[:, :])
    _split_multi_waits(nc)
    return nc


_WCOLS16 = {}
_WCOLS32 = {}


def kernel(**inputs):
    for l in range(1, 8):
        if np.abs(np.asarray(inputs[f"bt{l}"], np.float32)).max() > 0:
            return _numpy_forward(inputs)
        if np.asarray(inputs[f"g{l}"], np.float32).min() <= 0:
            return _numpy_forward(inputs)

    wpack, offs16, wpack32, offs32 = _build_consts(inputs)
    global _WCOLS16, _WCOLS32
    _WCOLS16 = {"s1h": 32, "s1l": 32}
    _WCOLS32 = {"s2": 128, "s3": 64, "s8": 64,
                "v6a": 64, "v6b": 64, "v7": 64}
    for l in range(4, 8):
        for suf in ("a", "b"):
            _WCOLS32[f"t{l}{suf}"] = 128

    x = np.asarray(inputs["x"], np.float32)
    xT = np.ascontiguousarray(x.T)               # [128, 524288]
    xhi = xT.astype(F16)
    xlo = (xT - xhi.astype(np.float32)).astype(F16)
    b8 = np.asarray(inputs["b8"], np.float32).reshape(1, 1)

    nc = _build_program(offs16, wpack.shape[1], offs32, wpack32.shape[1],
                        float(b8[0, 0]))

    in_maps = []
    for c in range(N_CORES):
        s = slice(c * RPC, (c + 1) * RPC)
        in_maps.append({
            "xhi": np.ascontiguousarray(xhi[:, s]),
            "xlo": np.ascontiguousarray(xlo[:, s]),
            "wpack": wpack, "wpack32": wpack32,
        })

    from concourse.bass_utils import run_bass_kernel_spmd
    res = run_bass_kernel_spmd(nc, in_maps, core_ids=list(range(N_CORES)))

    out = np.empty((ROWS, 1), np.float32)
    for c in range(N_CORES):
        out[c * RPC:(c + 1) * RPC, 0] = res.results[c]["out"].reshape(-1)
    return out
